# revision 42
# baseline (speedup 1.0000x reference)
"""Trainium2 Bass kernel for nn_BeliefModuleOld (segment_reduce).

Reference semantics per batch element b and treat type tt:
  valid[t] = (vision[b,t] != 0) and (max(visible_treats[b,t,tt,0:5]) > 0.5)
  out[b,tt,:] = visible_treats[b, last valid t, tt, :]  (or [0,0,0,0,0,1] if none)

Strategy: pure data-parallel over batch, 8 cores; default ring "v33"
(= v25 + cold-start fill trims: on rep 0, tile 0 is processed as four
~30-row chunks aligned to its four load cuts so the first
relu/tree/blend chain starts ~3.4 us into the run instead of ~12, and
tile 1 as two load-cut-aligned halves so its tree starts when half its
data lands instead of idling ~5 us for the full tile -- the
single-shot latency the harness grades includes this fill. Steady
state is byte-identical to v25, which won every paired A/B. The v31
early-v-load reorder measured 1-3 us slower and was dropped; a
matching drain-side split was rejected because concurrent subtile
stores reproduced v30's silent-corruption pattern).

Per core (125,000 rows = 8 x [128 part x 122 rows] tiles + [8 x 9] tail):
  - DMA: x tiles load via gpsimd/SWDGE in two f-halves (1.87 MB each) so
    the ACT relu can start on the first half while the second streams;
    v loads per tile-pair on the ACT HWDGE; o stores + the tail pass on
    the otherwise-idle SP HWDGE (a store trigger's wait on blend
    completion would stall the in-order ACT queue in front of the next
    relu). Total HBM traffic 38.5 MB/core against the ~358 GB/s
    HBM-per-NeuronCore cap = 107.5 us hard floor.
  - ACT: r = relu(x - 0.5) over the whole tile, fully contiguous APs
    (the d-major transposing layout halves ACT rate), output bf16. The
    affine runs in fp32 internally so the 0.5 threshold is f32-exact,
    and any positive f32 rounds to a positive bf16 => sign(r) exactly
    encodes (x > 0.5). ~5.4 us/tile, plus the out-tile default inits.
  - DVE (the scarce engine): 3-op pair max-tree over r in bf16 -- lvl1
    reads packed bf16 pairs so TensorTensor's 2x_1p perf mode engages
    (f32 TT is stuck at 1x; this tree is 3.7 us/tile vs 5.3 for f32) --
    then valid = (hm > 0) * v as one stt (1.3 us), then 5 ascending-t
    copy_predicated blends, last valid wins (8.3 us, no fast mode
    exists for cpred). ~13.2 us/tile, just under the DMA cadence.

Measured (device-resident reps-differencing, 8-core SPMD): ~112-117
us/core steady state vs 128.3 us for the prior tensor_max/f32 baseline;
DMA-only mode measures 107.4 us, so the kernel sits ~5-8% off the HBM
roofline. Rejected on measurement: cross-tile DVE interleaving (v7/v18),
d-major relu layouts (v22/v24), TSP-based trees (v6/v14 -- stt's 2x_2p
uop never engages on HW), u16 stt operands (v28), single-instruction
blends (v11/v15 -- 5D APs exceed the TENSOR3D codegen limit).
"""

import numpy as np

import concourse.bass as bass
import concourse.bacc as bacc
import concourse.tile as tile
from concourse import mybir
from concourse.alu_op_type import AluOpType
from concourse.bass_utils import run_bass_kernel_spmd

B, T, NT, D = 1_000_000, 5, 2, 6
NCORES = 8
BC = B // NCORES  # 125,000 per core
P = 125           # SBUF partitions used
F = 125           # batch elements per partition per tile
NTILES = BC // (P * F)  # 8 tiles, exact


def _copy_predicated(eng, out, mask, data):
    # Same as BassVectorEngine.copy_predicated but with opt=False lowering so
    # the three operand APs keep identical [p, f, nt, d] structure (the
    # broadcast mask AP cannot merge dims; unoptimized APs keep the sim's
    # np.where shapes aligned and the HW element streams in lockstep).
    return eng.add_instruction(
        mybir.InstCopyPredicated(
            name=f"I-{eng.bass.next_id()}",
            ins=[eng.lower_ap(mask, opt=False), eng.lower_ap(data, opt=False)],
            outs=[eng.lower_ap(out, opt=False)],
        )
    )


def build_nc(bc=BC, p=P, f=F, reps=1, mode="full", ring="v33"):
    ntiles = bc // (p * f)
    assert p * f * ntiles == bc, (bc, p, f)
    nc = bacc.Bacc("TRN2", target_bir_lowering=False)

    x = nc.dram_tensor("x", [bc, T, NT, D], mybir.dt.float32, kind="ExternalInput")
    v = nc.dram_tensor("v", [bc, T], mybir.dt.int32, kind="ExternalInput")
    o = nc.dram_tensor("o", [bc, NT, D], mybir.dt.float32, kind="ExternalOutput")

    # [ntiles, p, per-partition-contiguous block]
    xr = x[:].rearrange("(n p f) t nt d -> n p (f t nt d)", p=p, f=f)
    vr = v[:].rearrange("(n p f) t -> n p (f t)", p=p, f=f)
    orr = o[:].rearrange("(n p f) nt d -> n p (f nt d)", p=p, f=f)

    fdt = mybir.dt.float32

    if ring == "v3":
        # v2 with shorter fill/drain: v loaded per-pair just in time
        # (x0 owns the SDMA fill), o stored per tile (cuts the drain),
        # tail pass emitted mid-pipeline where DVE has slack.
        P8, q, nload = 128, 122, 8
        main = P8 * q * nload  # 124,928
        tp, tq = 8, 9          # tail 72 = 8 * 9
        assert main + tp * tq == bc
        npair = nload // 2
        xm = x[0:main].rearrange("(n p f) t nt d -> n p f t nt d", p=P8, f=q)
        vm = v[0:main].rearrange("(n p f) t -> p n (f t)", p=P8, f=q)
        om = o[0:main].rearrange("(n p f) nt d -> n p (f nt d)", p=P8, f=q)
        xt_d = x[main:bc].rearrange("(p f) t nt d -> p f t nt d", p=tp, f=tq)
        vt_d = v[main:bc].rearrange("(p f) t -> p (f t)", p=tp, f=tq)
        ot_d = o[main:bc].rearrange("(p f) nt d -> p (f nt d)", p=tp, f=tq)
        with tile.TileContext(nc) as tc:
            with (
                tc.tile_pool(name="xs", bufs=3) as xpool,
                tc.tile_pool(name="vs", bufs=2) as vpool,
                tc.tile_pool(name="os", bufs=3) as opool,
                tc.tile_pool(name="wk", bufs=2) as wpool,
            ):
                dflt = wpool.tile([P8, NT, D], fdt, tag="dflt", bufs=1)
                nc.gpsimd.memset(dflt[:, :, 0:5], 0.0)
                nc.gpsimd.memset(dflt[:, :, 5:6], 1.0)
                nbias = None
                if ring in ("v20", "v21"):
                    nbias = wpool.tile([P8, 1], fdt, tag="nbias", bufs=1)
                    nc.gpsimd.memset(nbias[:], -0.5)

                def compute(pp, ff, xt, vslice, ot, tg):
                    a = wpool.tile([pp, ff, T, NT], fdt, tag="a" + tg)
                    valid = wpool.tile(
                        [pp, ff, T, NT], mybir.dt.uint8, tag="va" + tg
                    )
                    if ring == "v4":
                        # single pool-max over the contiguous d<5 run --
                        # ~2x faster than the stride-6 tensor_max tree
                        nc.vector.pool_max(a[:], xt[:, :, :, :, 0:5])
                    elif ring == "v6":
                        # max tree via TensorScalarPtr (max(a,b) as
                        # (a*1) max b): stt supports the 2x_2p DVE perf
                        # mode for f32 SBUF operands, TensorTensor does
                        # not
                        bt = wpool.tile([pp, ff, T, NT], fdt, tag="b" + tg)

                        def stt_max(out, i0, i1):
                            nc.vector.scalar_tensor_tensor(
                                out=out, in0=i0, scalar=1.0, in1=i1,
                                op0=AluOpType.mult, op1=AluOpType.max,
                            )

                        stt_max(a[:], xt[:, :, :, :, 0], xt[:, :, :, :, 1])
                        stt_max(bt[:], xt[:, :, :, :, 2], xt[:, :, :, :, 3])
                        stt_max(a[:], a[:], bt[:])
                        stt_max(a[:], a[:], xt[:, :, :, :, 4])
                    else:
                        bt = wpool.tile([pp, ff, T, NT], fdt, tag="b" + tg)
                        nc.vector.tensor_max(
                            a[:], xt[:, :, :, :, 0], xt[:, :, :, :, 1]
                        )
                        nc.vector.tensor_max(
                            bt[:], xt[:, :, :, :, 2], xt[:, :, :, :, 3]
                        )
                        nc.vector.tensor_max(a[:], a[:], bt[:])
                        nc.vector.tensor_max(a[:], a[:], xt[:, :, :, :, 4])
                    vb = vslice.unsqueeze(3).broadcast_to((pp, ff, T, NT))
                    nc.vector.scalar_tensor_tensor(
                        out=valid[:], in0=a[:], scalar=0.5, in1=vb,
                        op0=AluOpType.is_gt, op1=AluOpType.mult,
                    )
                    for t in range(T):
                        mask = (
                            valid[:, :, t, :]
                            .unsqueeze(3)
                            .broadcast_to((pp, ff, NT, D))
                        )
                        _copy_predicated(nc.vector, ot, mask, xt[:, :, t, :, :])

                for r in range(reps):
                    vts = {}
                    ots = {}

                    def vload(jj):
                        vpair = vpool.tile([P8, 2, q, T], mybir.dt.int32, tag="v")
                        vts[jj] = vpair
                        nc.scalar.dma_start(
                            out=vpair[:].rearrange("p n f t -> p n (f t)"),
                            in_=vm[:, 2 * jj : 2 * jj + 2, :],
                        )

                    def tail_pass():
                        xtt = xpool.tile([tp, tq, T, NT, D], fdt, tag="xT", bufs=2)
                        nc.sync.dma_start(
                            out=xtt[:].rearrange("p f t nt d -> p (f t nt d)"),
                            in_=xt_d[:].rearrange("p f t nt d -> p (f t nt d)"),
                        )
                        vtt = vpool.tile([tp, tq, T], mybir.dt.int32, tag="vT", bufs=2)
                        ott = opool.tile([tp, tq, NT, D], fdt, tag="oT", bufs=2)
                        nc.sync.dma_start(
                            out=vtt[:].rearrange("p f t -> p (f t)"), in_=vt_d[:]
                        )
                        nc.scalar.copy(
                            ott[:],
                            dflt[0:tp].unsqueeze(1).broadcast_to((tp, tq, NT, D)),
                        )
                        compute(tp, tq, xtt, vtt[:], ott[:], "T")
                        nc.sync.dma_start(
                            out=ot_d[:],
                            in_=ott[:].rearrange("p f nt d -> p (f nt d)"),
                        )

                    for k in range(nload):
                        j, kk = k // 2, k % 2
                        xt = xpool.tile([P8, q, T, NT, D], fdt, tag="x")
                        nc.gpsimd.dma_start(
                            out=xt[:].rearrange("p f t nt d -> p (f t nt d)"),
                            in_=xm[k].rearrange("p f t nt d -> p (f t nt d)"),
                        )
                        if kk == 0 and mode != "dma":
                            # v for this pair (j=0) or the next (lookahead)
                            for jj in [0, 1] if j == 0 else [j + 1]:
                                if jj < npair:
                                    vload(jj)
                            # init out tiles one tile ahead of the store-wait
                            for kx in (
                                [k, k + 1, k + 2] if k == 0 else [k + 1, k + 2]
                            ):
                                if kx < nload and kx not in ots:
                                    oti = opool.tile(
                                        [P8, q, NT, D], fdt, tag="o"
                                    )
                                    ots[kx] = oti
                                    nc.scalar.copy(
                                        oti[:],
                                        dflt[:]
                                        .unsqueeze(1)
                                        .broadcast_to((P8, q, NT, D)),
                                    )
                        if mode == "dma":
                            if kk == 0 and j == 0:
                                vload(0)
                                vload(1)
                            elif kk == 0:
                                vload(j + 1) if j + 1 < npair else None
                            nc.scalar.dma_start(
                                out=om[k],
                                in_=xt[:].rearrange("p f t nt d -> p (f t nt d)")[
                                    :, 0 : q * NT * D
                                ],
                            )
                            if k == 4:
                                xtt = xpool.tile(
                                    [tp, tq, T, NT, D], fdt, tag="xT", bufs=2
                                )
                                nc.sync.dma_start(
                                    out=xtt[:].rearrange(
                                        "p f t nt d -> p (f t nt d)"
                                    ),
                                    in_=xt_d[:].rearrange(
                                        "p f t nt d -> p (f t nt d)"
                                    ),
                                )
                            continue
                        compute(P8, q, xt, vts[j][:, kk], ots[k][:], "")
                        nc.scalar.dma_start(
                            out=om[k],
                            in_=ots[k][:].rearrange("p f nt d -> p (f nt d)"),
                        )
                        if k == 4:
                            tail_pass()
        nc.compile()
        return nc

    if ring.startswith("mi_"):
        # microbench: per-op-class DVE/ACT throughput on real tile shapes.
        # One resident x tile; each rep issues one tile's worth of the op
        # class with double-buffered outputs so reps measure streaming
        # throughput (not drain latency).
        kind = ring[3:]
        P8, q = 128, 122
        bf16 = mybir.dt.bfloat16
        nft = q * T * NT
        xm0 = x[0 : P8 * q].rearrange("(p f) t nt d -> p (f t nt d)", p=P8, f=q)
        vm0 = v[0 : P8 * q].rearrange("(p f) t -> p (f t)", p=P8, f=q)
        om0 = o[0 : P8 * q].rearrange("(p f) nt d -> p (f nt d)", p=P8, f=q)
        with tile.TileContext(nc) as tc:
            with (
                tc.tile_pool(name="xs", bufs=1) as xpool,
                tc.tile_pool(name="wk", bufs=2) as wpool,
            ):
                nbias = wpool.tile([P8, 1], fdt, tag="nbias", bufs=1)
                nc.gpsimd.memset(nbias[:], -0.5)
                xt = xpool.tile([P8, q, T, NT, D], fdt, tag="x", bufs=1)
                nc.gpsimd.dma_start(
                    out=xt[:].rearrange("p f t nt d -> p (f t nt d)"), in_=xm0
                )
                vt = xpool.tile([P8, q, T], mybir.dt.int32, tag="v", bufs=1)
                nc.scalar.dma_start(
                    out=vt[:].rearrange("p f t -> p (f t)"), in_=vm0
                )
                rt = xpool.tile([P8, 5, nft], bf16, tag="r", bufs=1)
                nc.scalar.activation(
                    out=rt[:],
                    in_=xt[:, :, :, :, 0:5].rearrange("p f t nt d -> p d (f t nt)"),
                    func=mybir.ActivationFunctionType.Relu,
                    bias=nbias[:],
                )
                vfix = xpool.tile([P8, q, T, NT], mybir.dt.uint8, tag="vf", bufs=1)
                nc.gpsimd.memset(vfix[:], 1)
                ot = xpool.tile([P8, q, NT, D], fdt, tag="o", bufs=1)
                nc.gpsimd.memset(ot[:], 0.0)
                r6n = None
                if kind == "treen":
                    r6n = xpool.tile([P8, q, T, NT, D], bf16, tag="r6res", bufs=1)
                    nc.scalar.activation(
                        out=r6n[:].rearrange("p f t nt d -> p (f t nt d)"),
                        in_=xt[:].rearrange("p f t nt d -> p (f t nt d)"),
                        func=mybir.ActivationFunctionType.Relu,
                        bias=nbias[:],
                    )
                for r in range(reps):
                    if kind == "treef32":
                        a2 = wpool.tile([P8, q, T, NT, 2], fdt, tag="a2")
                        a = wpool.tile([P8, q, T, NT], fdt, tag="a")
                        nc.vector.tensor_max(
                            a2[:], xt[:, :, :, :, 0:2], xt[:, :, :, :, 2:4]
                        )
                        nc.vector.tensor_max(a[:], a2[:, :, :, :, 0], a2[:, :, :, :, 1])
                        nc.vector.tensor_max(a[:], a[:], xt[:, :, :, :, 4])
                    elif kind == "treebf16":
                        m2 = wpool.tile([P8, 2, nft], bf16, tag="m2")
                        hm = wpool.tile([P8, nft], bf16, tag="hm")
                        nc.vector.tensor_max(
                            m2[:].rearrange("p a b -> p (a b)"),
                            rt[:, 0:2, :].rearrange("p a b -> p (a b)"),
                            rt[:, 2:4, :].rearrange("p a b -> p (a b)"),
                        )
                        nc.vector.tensor_max(hm[:], m2[:, 0], m2[:, 1])
                        nc.vector.tensor_max(hm[:], hm[:], rt[:, 4, :])
                    elif kind == "cpred5":
                        for t in range(T):
                            mask = (
                                vfix[:, :, t, :]
                                .unsqueeze(3)
                                .broadcast_to((P8, q, NT, D))
                            )
                            _copy_predicated(nc.vector, ot[:], mask, xt[:, :, t, :, :])
                    elif kind == "stt":
                        va = wpool.tile([P8, q, T, NT], mybir.dt.uint8, tag="va")
                        vb = vt[:].unsqueeze(3).broadcast_to((P8, q, T, NT))
                        nc.vector.scalar_tensor_tensor(
                            out=va[:], in0=xt[:, :, :, :, 0], scalar=0.5, in1=vb,
                            op0=AluOpType.is_gt, op1=AluOpType.mult,
                        )
                    elif kind == "reluc6":
                        # fully contiguous: relu over all 6 d, natural layout
                        r6 = wpool.tile([P8, q, T, NT, D], bf16, tag="r6")
                        nc.scalar.activation(
                            out=r6[:].rearrange("p f t nt d -> p (f t nt d)"),
                            in_=xt[:].rearrange("p f t nt d -> p (f t nt d)"),
                            func=mybir.ActivationFunctionType.Relu,
                            bias=nbias[:],
                        )
                    elif kind == "relud2":
                        # d-major out, but iterate d fastest (contig 20B reads)
                        rr2 = wpool.tile([P8, 5, nft], bf16, tag="rr2")
                        nc.scalar.activation(
                            out=rr2[:].rearrange("p d n -> p n d"),
                            in_=xt[:, :, :, :, 0:5].rearrange(
                                "p f t nt d -> p (f t nt) d"
                            ),
                            func=mybir.ActivationFunctionType.Relu,
                            bias=nbias[:],
                        )
                    elif kind == "treen":
                        # natural-layout bf16 tree (lvl1 packed 2x, rest 1x)
                        m2n = wpool.tile([P8, q, T, NT, 2], bf16, tag="m2n")
                        hmn = wpool.tile([P8, q, T, NT], bf16, tag="hmn")
                        nc.vector.tensor_max(
                            m2n[:], r6n[:, :, :, :, 0:2], r6n[:, :, :, :, 2:4]
                        )
                        nc.vector.tensor_max(
                            hmn[:], m2n[:, :, :, :, 0], m2n[:, :, :, :, 1]
                        )
                        nc.vector.tensor_max(hmn[:], hmn[:], r6n[:, :, :, :, 4])
                    elif kind == "relu":
                        rr = wpool.tile([P8, 5, nft], bf16, tag="rr")
                        nc.scalar.activation(
                            out=rr[:],
                            in_=xt[:, :, :, :, 0:5].rearrange(
                                "p f t nt d -> p d (f t nt)"
                            ),
                            func=mybir.ActivationFunctionType.Relu,
                            bias=nbias[:],
                        )
                    elif kind == "initcopy":
                        oc = wpool.tile([P8, q, NT, D], fdt, tag="oc")
                        nc.scalar.copy(
                            oc[:],
                            ot[:, 0:1].broadcast_to((P8, q, NT, D)),
                        )
                    else:
                        raise ValueError(kind)
                nc.sync.dma_start(out=om0, in_=ot[:].rearrange("p f nt d -> p (f nt d)"))
        nc.compile()
        return nc

    if ring in ("v22", "v23", "v24", "v25", "v26", "v27", "v28", "v29", "v30", "v31", "v32", "v33"):
        # v12's straight-line structure + the ACT-threshold/bf16-tree trick,
        # tuned so the in-order ACT queue never parks a semaphore wait in
        # front of work the DVE needs next:
        #   - ACT per tile: r(k) = relu(x - 0.5) -> bf16 d-major planes
        #     (fp32-internal affine keeps the 0.5 threshold f32-exact;
        #     positive f32 rounds to positive bf16), THEN the next pair's
        #     v-load trigger + out-init copy.
        #   - DVE per tile: 3 contiguous bf16 tensor_max (2x_1p perf mode,
        #     half the cycles of the f32 tree) + stt valid + 5 blends.
        #   - o-stores ride the SP queue (their waits on blend completion
        #     would stall ACT); tail loads prefetch at k=0 ahead of them.
        P8, q, nload = 128, 122, 8
        main = P8 * q * nload  # 124,928
        tp, tq = 8, 9          # tail 72 = 8 * 9
        assert main + tp * tq == bc
        npair = nload // 2
        bf16 = mybir.dt.bfloat16
        xm = x[0:main].rearrange("(n p f) t nt d -> n p f t nt d", p=P8, f=q)
        vm = v[0:main].rearrange("(n p f) t -> p n (f t)", p=P8, f=q)
        om = o[0:main].rearrange("(j n p f) nt d -> j p n (f nt d)", p=P8, f=q, n=2)
        om1 = o[0:main].rearrange("(n p f) nt d -> n p (f nt d)", p=P8, f=q)
        xt_d = x[main:bc].rearrange("(p f) t nt d -> p f t nt d", p=tp, f=tq)
        vt_d = v[main:bc].rearrange("(p f) t -> p (f t)", p=tp, f=tq)
        ot_d = o[main:bc].rearrange("(p f) nt d -> p (f nt d)", p=tp, f=tq)
        with tile.TileContext(nc) as tc:
            vb4 = 4 if ring == "v27" else 3
            with (
                tc.tile_pool(name="xs", bufs=3) as xpool,
                tc.tile_pool(name="vs", bufs=vb4) as vpool,
                tc.tile_pool(name="os", bufs=3) as opool,
                tc.tile_pool(name="wk", bufs=2) as wpool,
            ):
                dflt = wpool.tile([P8, NT, D], fdt, tag="dflt", bufs=1)
                nc.gpsimd.memset(dflt[:, :, 0:5], 0.0)
                nc.gpsimd.memset(dflt[:, :, 5:6], 1.0)
                nbias = wpool.tile([P8, 1], fdt, tag="nbias", bufs=1)
                nc.gpsimd.memset(nbias[:], -0.5)

                def emit_r(pp, ff, xt, tg, par=0, nsp=None):
                    nft = ff * T * NT
                    if ring == "v24":
                        # alternate per tile parity: even tiles contiguous
                        # relu + natural tree; odd tiles d-major relu
                        # (slower on ACT) + plane tree (faster on DVE) --
                        # balances ACT ~16us/pair vs DVE ~24us/pair
                        if par == 0:
                            # bufs=1 per parity tag: alternation already
                            # gives a 2-tile pipeline between ACT and DVE
                            rt = wpool.tile(
                                [pp, ff, T, NT, D], bf16, tag="rA" + tg, bufs=1
                            )
                            nc.scalar.activation(
                                out=rt[:].rearrange("p f t nt d -> p (f t nt d)"),
                                in_=xt[:].rearrange("p f t nt d -> p (f t nt d)"),
                                func=mybir.ActivationFunctionType.Relu,
                                bias=nbias[0:pp],
                            )
                        else:
                            rt = wpool.tile(
                                [pp, 5, nft], bf16, tag="rB" + tg, bufs=1
                            )
                            nc.scalar.activation(
                                out=rt[:],
                                in_=xt[:, :, :, :, 0:5].rearrange(
                                    "p f t nt d -> p d (f t nt)"
                                ),
                                func=mybir.ActivationFunctionType.Relu,
                                bias=nbias[0:pp],
                            )
                        return rt
                    if ring in ("v23", "v25", "v26", "v27", "v28", "v29", "v30", "v31", "v32", "v33"):
                        # fully contiguous relu (both APs flat) runs ~2x the
                        # rate of the d-major transposing write; the tree
                        # then runs in natural layout (lvl1 still packed-2x).
                        # v25 splits the relu in two f-halves so the first
                        # half starts as soon as its half of x lands.
                        rt = wpool.tile([pp, ff, T, NT, D], bf16, tag="r" + tg)
                        if nsp is not None and ff >= nsp:
                            s = ff // nsp
                            cuts = [i * s for i in range(nsp)] + [ff]
                            halves = list(zip(cuts[:-1], cuts[1:]))
                        elif ring in ("v25", "v27", "v28", "v29", "v30", "v31", "v32", "v33") and ff > 1:
                            halves = [(0, ff // 2), (ff // 2, ff)]
                        elif ring == "v26" and ff >= 4:
                            s = ff // 4
                            halves = [
                                (0, s), (s, 2 * s), (2 * s, 3 * s), (3 * s, ff)
                            ]
                        else:
                            halves = [(0, ff)]
                        for lo, hi in halves:
                            nc.scalar.activation(
                                out=rt[:, lo:hi].rearrange(
                                    "p f t nt d -> p (f t nt d)"
                                ),
                                in_=xt[:, lo:hi].rearrange(
                                    "p f t nt d -> p (f t nt d)"
                                ),
                                func=mybir.ActivationFunctionType.Relu,
                                bias=nbias[0:pp],
                            )
                        return rt
                    rt = wpool.tile([pp, 5, nft], bf16, tag="r" + tg)
                    nc.scalar.activation(
                        out=rt[:],
                        in_=xt[:, :, :, :, 0:5].rearrange(
                            "p f t nt d -> p d (f t nt)"
                        ),
                        func=mybir.ActivationFunctionType.Relu,
                        bias=nbias[0:pp],
                    )
                    return rt

                def compute(pp, ff, xt, rt, vslice, ot, tg):
                    nft = ff * T * NT
                    if ring == "v28":
                        # stt hits 2x_1p only when every operand is 2-byte
                        # packed: expand v to u16 [p,f,T,NT] on ACT (slack)
                        # so valid = (hm > 0) * vx runs at 2 elem/cycle
                        valid = wpool.tile(
                            [pp, ff, T, NT], mybir.dt.uint16, tag="va" + tg
                        )
                        vx = wpool.tile(
                            [pp, ff, T, NT], mybir.dt.uint16, tag="vx" + tg
                        )
                        nc.scalar.copy(
                            vx[:],
                            vslice.unsqueeze(3).broadcast_to((pp, ff, T, NT)),
                        )
                    else:
                        valid = wpool.tile(
                            [pp, ff, T, NT], mybir.dt.uint8, tag="va" + tg
                        )
                    hm = wpool.tile([pp, ff, T, NT], bf16, tag="hm" + tg, bufs=1)
                    hmf = hm[:].rearrange("p f t nt -> p (f t nt)")
                    if ring == "v24":
                        natural = len(rt.shape) == 5
                    if ring == "v24" and natural:
                        m2 = wpool.tile(
                            [pp, ff, T, NT, 2], bf16, tag="m2A" + tg, bufs=1
                        )
                        nc.vector.tensor_max(
                            m2[:], rt[:, :, :, :, 0:2], rt[:, :, :, :, 2:4]
                        )
                        nc.vector.tensor_max(
                            hm[:], m2[:, :, :, :, 0], m2[:, :, :, :, 1]
                        )
                        nc.vector.tensor_max(hm[:], hm[:], rt[:, :, :, :, 4])
                    elif ring == "v24":
                        m2 = wpool.tile(
                            [pp, 2, nft], bf16, tag="m2B" + tg, bufs=1
                        )
                        nc.vector.tensor_max(
                            m2[:].rearrange("p a b -> p (a b)"),
                            rt[:, 0:2, :].rearrange("p a b -> p (a b)"),
                            rt[:, 2:4, :].rearrange("p a b -> p (a b)"),
                        )
                        nc.vector.tensor_max(hmf, m2[:, 0], m2[:, 1])
                        nc.vector.tensor_max(hmf, hmf, rt[:, 4, :])
                    elif ring in ("v23", "v25", "v26", "v27", "v28", "v29", "v30", "v31", "v32", "v33"):
                        m2 = wpool.tile(
                            [pp, ff, T, NT, 2], bf16, tag="m2" + tg, bufs=1
                        )
                        nc.vector.tensor_max(
                            m2[:], rt[:, :, :, :, 0:2], rt[:, :, :, :, 2:4]
                        )
                        nc.vector.tensor_max(
                            hm[:], m2[:, :, :, :, 0], m2[:, :, :, :, 1]
                        )
                        nc.vector.tensor_max(hm[:], hm[:], rt[:, :, :, :, 4])
                    else:
                        m2 = wpool.tile(
                            [pp, 2, nft], bf16, tag="m2" + tg, bufs=1
                        )
                        nc.vector.tensor_max(
                            m2[:].rearrange("p a b -> p (a b)"),
                            rt[:, 0:2, :].rearrange("p a b -> p (a b)"),
                            rt[:, 2:4, :].rearrange("p a b -> p (a b)"),
                        )
                        nc.vector.tensor_max(hmf, m2[:, 0], m2[:, 1])
                        nc.vector.tensor_max(hmf, hmf, rt[:, 4, :])
                    if ring == "v28":
                        nc.vector.scalar_tensor_tensor(
                            out=valid[:], in0=hm[:], scalar=0.0, in1=vx[:],
                            op0=AluOpType.is_gt, op1=AluOpType.mult,
                        )
                    else:
                        vb = vslice.unsqueeze(3).broadcast_to((pp, ff, T, NT))
                        nc.vector.scalar_tensor_tensor(
                            out=valid[:], in0=hm[:], scalar=0.0, in1=vb,
                            op0=AluOpType.is_gt, op1=AluOpType.mult,
                        )
                    for t in range(T):
                        mask = (
                            valid[:, :, t, :]
                            .unsqueeze(3)
                            .broadcast_to((pp, ff, NT, D))
                        )
                        _copy_predicated(nc.vector, ot, mask, xt[:, :, t, :, :])

                for r in range(reps):
                    vts = {}
                    ots = {}
                    for k in range(nload):
                        j, kk = k // 2, k % 2
                        xt = xpool.tile([P8, q, T, NT, D], fdt, tag="x")
                        if ring in ("v25", "v26", "v27", "v28", "v29", "v30", "v31", "v32", "v33"):
                            nsp = 4 if ring == "v26" else 2
                            if ring in ("v31", "v32") and k == 0 and r == 0:
                                # quarter-split the first tile so the fill
                                # (x -> relu -> tree) starts sooner; steady
                                # state keeps the measured-best halves
                                nsp = 4
                            s = q // nsp
                            cuts = [i * s for i in range(nsp)] + [q]
                            if ring == "v33" and k == 0 and r == 0:
                                # quarters aligned to the half-tile compute
                                # boundary (61) so neither half's relu waits
                                # on the other half's load
                                cuts = [0, 30, 61, 91, q]
                            for lo, hi in zip(cuts[:-1], cuts[1:]):
                                nc.gpsimd.dma_start(
                                    out=xt[:, lo:hi].rearrange(
                                        "p f t nt d -> p (f t nt d)"
                                    ),
                                    in_=xm[k][:, lo:hi].rearrange(
                                        "p f t nt d -> p (f t nt d)"
                                    ),
                                )
                        else:
                            nc.gpsimd.dma_start(
                                out=xt[:].rearrange("p f t nt d -> p (f t nt d)"),
                                in_=xm[k].rearrange("p f t nt d -> p (f t nt d)"),
                            )
                        if k == 0:
                            # tail loads first on SP, ahead of store waits
                            xtt = xpool.tile(
                                [tp, tq, T, NT, D], fdt, tag="xT", bufs=2
                            )
                            nc.sync.dma_start(
                                out=xtt[:].rearrange("p f t nt d -> p (f t nt d)"),
                                in_=xt_d[:].rearrange("p f t nt d -> p (f t nt d)"),
                            )
                            if mode != "dma":
                                vtt = vpool.tile(
                                    [tp, tq, T], mybir.dt.int32, tag="vT", bufs=2
                                )
                                ott = opool.tile(
                                    [tp, tq, NT, D], fdt, tag="oT", bufs=2
                                )
                                nc.sync.dma_start(
                                    out=vtt[:].rearrange("p f t -> p (f t)"),
                                    in_=vt_d[:],
                                )
                        newpairs = []
                        if kk == 0:
                            newpairs = [
                                jj
                                for jj in ([0, 1] if j == 0 else [j + 1])
                                if jj < npair
                            ]
                        if ring == "v31":
                            # v-load triggers ahead of the relu: they don't
                            # wait on x(k), so the DMAs flow during the x
                            # load instead of queuing behind r(k)'s wait.
                            # (Unlike v29, the 0.7us init copies stay AFTER
                            # the relu -- moving those early cost 7us.)
                            for jj in newpairs:
                                vpair = vpool.tile(
                                    [P8, 2, q, T], mybir.dt.int32, tag="v"
                                )
                                vts[jj] = vpair
                                nc.scalar.dma_start(
                                    out=vpair[:].rearrange(
                                        "p n f t -> p n (f t)"
                                    ),
                                    in_=vm[:, 2 * jj : 2 * jj + 2, :],
                                )
                        rt = None
                        v33split = ring == "v33" and k <= 1 and r == 0
                        if mode != "dma" and ring != "v29" and not v33split:
                            rnsp = 4 if (ring in ("v31", "v32") and k == 0 and r == 0) else None
                            rt = emit_r(P8, q, xt, "", par=kk, nsp=rnsp)
                        for jj in newpairs:
                            if ring != "v31":
                                vpair = vpool.tile(
                                    [P8, 2, q, T], mybir.dt.int32, tag="v"
                                )
                                vts[jj] = vpair
                                nc.scalar.dma_start(
                                    out=vpair[:].rearrange(
                                        "p n f t -> p n (f t)"
                                    ),
                                    in_=vm[:, 2 * jj : 2 * jj + 2, :],
                                )
                            if mode == "dma":
                                continue
                            opair = opool.tile(
                                [P8, 2, q, NT, D], fdt, tag="o"
                            )
                            ots[jj] = opair
                            nc.scalar.copy(
                                opair[:],
                                dflt[:]
                                .unsqueeze(1)
                                .unsqueeze(1)
                                .broadcast_to((P8, 2, q, NT, D)),
                            )
                        if k == 0 and mode != "dma":
                            nc.scalar.copy(
                                ott[:],
                                dflt[0:tp]
                                .unsqueeze(1)
                                .broadcast_to((tp, tq, NT, D)),
                            )
                        if (
                            k == 0 and r == 0 and ring == "v33"
                            and mode != "dma"
                        ):
                            # cold run: the tail's ~2us of DVE work fits in
                            # the dead window before tile 0's first chunk
                            # lands (its SP loads arrive ~0.4us in); later
                            # reps keep the k==4 mid-pipeline placement
                            rtt = emit_r(tp, tq, xtt, "T")
                            compute(tp, tq, xtt, rtt, vtt[:], ott[:], "T")
                            nc.sync.dma_start(
                                out=ot_d[:],
                                in_=ott[:].rearrange("p f nt d -> p (f nt d)"),
                            )
                        if mode != "dma" and ring == "v29":
                            # relu emitted after the init/vload block: those
                            # don't wait on x(k), so ACT does them during the
                            # x DMA instead of queuing behind r(k)'s wait
                            rt = emit_r(P8, q, xt, "", par=kk)
                        if mode == "dma":
                            if kk == 1:
                                nc.sync.dma_start(
                                    out=om[j],
                                    in_=xt[:]
                                    .rearrange("p f t nt d -> p (f t nt d)")[
                                        :, 0 : 2 * q * NT * D
                                    ]
                                    .rearrange("p (n e) -> p n e", n=2),
                                )
                            continue
                        if v33split:
                            # cold-start fill trim (rep 0 only; steady state
                            # unchanged): tile 0 as four ~q/4 chunks aligned
                            # to its load cuts so the first relu/tree/blend
                            # chain starts ~3.4us in instead of ~12; tile 1
                            # as two halves (already load-cut-aligned) so
                            # its tree starts when half the data lands
                            # instead of idling ~5us for the full tile
                            ksplits = (
                                ((0, 30), (30, 61), (61, 91), (91, q))
                                if k == 0
                                else ((0, q // 2), (q // 2, q))
                            )
                            for lo, hi in ksplits:
                                xh = xt[:, lo:hi]
                                rh = emit_r(P8, hi - lo, xh, "H", nsp=1)
                                compute(
                                    P8, hi - lo, xh, rh,
                                    vts[j][:, kk, lo:hi],
                                    ots[j][:, kk, lo:hi], "H",
                                )
                        else:
                            compute(P8, q, xt, rt, vts[j][:, kk], ots[j][:, kk], "")
                        if ring == "v30" or j >= npair - 1:
                            # store each tile single, right after its blends
                            nc.sync.dma_start(
                                out=om1[k],
                                in_=ots[j][:, kk].rearrange(
                                    "p f nt d -> p (f nt d)"
                                ),
                            )
                        elif kk == 1:
                            nc.sync.dma_start(
                                out=om[j],
                                in_=ots[j][:].rearrange(
                                    "p n f nt d -> p n (f nt d)"
                                ),
                            )
                        if k == 4 and not (r == 0 and ring == "v33"):
                            # tail compute mid-pipeline where DVE has slack
                            rtt = emit_r(tp, tq, xtt, "T")
                            compute(tp, tq, xtt, rtt, vtt[:], ott[:], "T")
                            nc.sync.dma_start(
                                out=ot_d[:],
                                in_=ott[:].rearrange("p f nt d -> p (f nt d)"),
                            )
        nc.compile()
        return nc

    if ring in ("v8", "v12", "v14", "v15", "v20", "v21"):
        # v2 + fill/drain trims: the 2.5 MB v-load is split in half (x0
        # owns the SDMA fill; second half lands during pair 1), the
        # tail pass runs mid-pipeline (k==4) where DVE has slack, and
        # the final pair is stored as two 0.75 MB singles so the last
        # store starts one blend earlier.
        P8, q, nload = 128, 122, 8
        main = P8 * q * nload  # 124,928
        tp, tq = 8, 9          # tail 72 = 8 * 9
        assert main + tp * tq == bc
        npair = nload // 2
        xm = x[0:main].rearrange("(n p f) t nt d -> n p f t nt d", p=P8, f=q)
        vm = v[0:main].rearrange("(n p f) t -> p n (f t)", p=P8, f=q)
        om = o[0:main].rearrange("(j n p f) nt d -> j p n (f nt d)", p=P8, f=q, n=2)
        om1 = o[0:main].rearrange("(n p f) nt d -> n p (f nt d)", p=P8, f=q)
        xt_d = x[main:bc].rearrange("(p f) t nt d -> p f t nt d", p=tp, f=tq)
        vt_d = v[main:bc].rearrange("(p f) t -> p (f t)", p=tp, f=tq)
        ot_d = o[main:bc].rearrange("(p f) nt d -> p (f nt d)", p=tp, f=tq)
        with tile.TileContext(nc) as tc:
            with (
                tc.tile_pool(name="xs", bufs=3) as xpool,
                tc.tile_pool(name="vs", bufs=2) as vpool,
                tc.tile_pool(name="os", bufs=3) as opool,
                tc.tile_pool(name="wk", bufs=2) as wpool,
            ):
                dflt = wpool.tile([P8, NT, D], fdt, tag="dflt", bufs=1)
                nc.gpsimd.memset(dflt[:, :, 0:5], 0.0)
                nc.gpsimd.memset(dflt[:, :, 5:6], 1.0)
                nbias = None
                if ring in ("v20", "v21"):
                    nbias = wpool.tile([P8, 1], fdt, tag="nbias", bufs=1)
                    nc.gpsimd.memset(nbias[:], -0.5)

                def compute(pp, ff, xt, vslice, ot, tg):
                    valid = wpool.tile(
                        [pp, ff, T, NT], mybir.dt.uint8, tag="va" + tg
                    )
                    if ring not in ("v20", "v21"):
                        a = wpool.tile([pp, ff, T, NT], fdt, tag="a" + tg)
                    if ring in ("v20", "v21"):
                        # Threshold on ACT: r = relu(x - 0.5) computed in
                        # fp32 internally (threshold stays f32-exact; any
                        # positive f32 rounds to a positive bf16), written
                        # as bf16 d-major planes so the DVE max tree runs
                        # on packed contiguous bf16 -- TensorTensor's
                        # 2x_1p perf mode engages (f32 TT is stuck at 1x).
                        # valid = (max_d r > 0) * v replaces the old
                        # (max_d x > 0.5) * v.
                        nft = ff * T * NT
                        rt = wpool.tile(
                            [pp, 5, nft], mybir.dt.bfloat16, tag="r" + tg
                        )
                        nc.scalar.activation(
                            out=rt[:],
                            in_=xt[:, :, :, :, 0:5].rearrange(
                                "p f t nt d -> p d (f t nt)"
                            ),
                            func=mybir.ActivationFunctionType.Relu,
                            bias=nbias[0:pp],
                        )
                        m2 = wpool.tile(
                            [pp, 2, nft], mybir.dt.bfloat16, tag="m2" + tg,
                            bufs=1,
                        )
                        hm = wpool.tile(
                            [pp, ff, T, NT], mybir.dt.bfloat16, tag="hm" + tg,
                            bufs=1,
                        )
                        hmf = hm[:].rearrange("p f t nt -> p (f t nt)")
                        nc.vector.tensor_max(
                            m2[:].rearrange("p a b -> p (a b)"),
                            rt[:, 0:2, :].rearrange("p a b -> p (a b)"),
                            rt[:, 2:4, :].rearrange("p a b -> p (a b)"),
                        )
                        nc.vector.tensor_max(hmf, m2[:, 0], m2[:, 1])
                        nc.vector.tensor_max(hmf, hmf, rt[:, 4, :])
                        vb = vslice.unsqueeze(3).broadcast_to((pp, ff, T, NT))
                        nc.vector.scalar_tensor_tensor(
                            out=valid[:], in0=hm[:], scalar=0.0, in1=vb,
                            op0=AluOpType.is_gt, op1=AluOpType.mult,
                        )
                        for t in range(T):
                            mask = (
                                valid[:, :, t, :]
                                .unsqueeze(3)
                                .broadcast_to((pp, ff, NT, D))
                            )
                            _copy_predicated(nc.vector, ot, mask, xt[:, :, t, :, :])
                        return
                    if ring in ("v14", "v15"):
                        # max tree via TensorScalarPtr: TSP has 2x_2p/4x_2p
                        # uops (TensorTensor caps at 2x_1p, useless for f32),
                        # so f32 SBUF stt runs at 2 elem/cycle
                        bt = wpool.tile([pp, ff, T, NT], fdt, tag="b" + tg)

                        def stt_max(out, i0, i1):
                            nc.vector.scalar_tensor_tensor(
                                out=out, in0=i0, scalar=1.0, in1=i1,
                                op0=AluOpType.mult, op1=AluOpType.max,
                            )

                        stt_max(a[:], xt[:, :, :, :, 0], xt[:, :, :, :, 1])
                        stt_max(bt[:], xt[:, :, :, :, 2], xt[:, :, :, :, 3])
                        stt_max(a[:], a[:], bt[:])
                        stt_max(a[:], a[:], xt[:, :, :, :, 4])
                    elif ring == "v12":
                        a2 = wpool.tile([pp, ff, T, NT, 2], fdt, tag="a2" + tg)
                        nc.vector.tensor_max(
                            a2[:], xt[:, :, :, :, 0:2], xt[:, :, :, :, 2:4]
                        )
                        nc.vector.tensor_max(
                            a[:], a2[:, :, :, :, 0], a2[:, :, :, :, 1]
                        )
                        nc.vector.tensor_max(a[:], a[:], xt[:, :, :, :, 4])
                    else:
                        bt = wpool.tile([pp, ff, T, NT], fdt, tag="b" + tg)
                        nc.vector.tensor_max(
                            a[:], xt[:, :, :, :, 0], xt[:, :, :, :, 1]
                        )
                        nc.vector.tensor_max(
                            bt[:], xt[:, :, :, :, 2], xt[:, :, :, :, 3]
                        )
                        nc.vector.tensor_max(a[:], a[:], bt[:])
                        nc.vector.tensor_max(a[:], a[:], xt[:, :, :, :, 4])
                    vb = vslice.unsqueeze(3).broadcast_to((pp, ff, T, NT))
                    nc.vector.scalar_tensor_tensor(
                        out=valid[:], in0=a[:], scalar=0.5, in1=vb,
                        op0=AluOpType.is_gt, op1=AluOpType.mult,
                    )
                    if ring == "v15":
                        # single-instruction blend: out AP broadcast over t
                        # (stride 0); ascending-t same-address writes commit
                        # in order, so the last valid t wins
                        maskT = valid[:].unsqueeze(4).broadcast_to(
                            (pp, ff, T, NT, D)
                        )
                        outT = ot.unsqueeze(2).broadcast_to((pp, ff, T, NT, D))
                        _copy_predicated(nc.vector, outT, maskT, xt[:])
                    else:
                        for t in range(T):
                            mask = (
                                valid[:, :, t, :]
                                .unsqueeze(3)
                                .broadcast_to((pp, ff, NT, D))
                            )
                            _copy_predicated(nc.vector, ot, mask, xt[:, :, t, :, :])

                for r in range(reps):
                    vt = vpool.tile(
                        [P8, nload, q, T], mybir.dt.int32, tag="v",
                        bufs=1 if ring in ("v20", "v21") else 2,
                    )
                    ots = {}
                    for k in range(nload):
                        j, kk = k // 2, k % 2
                        if kk == 0 and mode != "dma":
                            for jj in [0, 1] if j == 0 else [j + 1]:
                                if jj >= npair:
                                    continue
                                opair = opool.tile(
                                    [P8, 2, q, NT, D], fdt, tag="o"
                                )
                                ots[jj] = opair
                                nc.scalar.copy(
                                    ots[jj][:],
                                    dflt[:]
                                    .unsqueeze(1)
                                    .unsqueeze(1)
                                    .broadcast_to((P8, 2, q, NT, D)),
                                )
                        xt = xpool.tile([P8, q, T, NT, D], fdt, tag="x")
                        nc.gpsimd.dma_start(
                            out=xt[:].rearrange("p f t nt d -> p (f t nt d)"),
                            in_=xm[k].rearrange("p f t nt d -> p (f t nt d)"),
                        )
                        if k == 0:
                            nc.scalar.dma_start(
                                out=vt[:, 0:4].rearrange("p n f t -> p n (f t)"),
                                in_=vm[:, 0:4, :],
                            )
                        elif k == 2:
                            nc.scalar.dma_start(
                                out=vt[:, 4:8].rearrange("p n f t -> p n (f t)"),
                                in_=vm[:, 4:8, :],
                            )
                        if k == 0 and ring == "v21":
                            # tail loads up front, before any store trigger
                            # parks a semaphore wait on the in-order SP queue
                            xtt = xpool.tile(
                                [tp, tq, T, NT, D], fdt, tag="xT", bufs=2
                            )
                            nc.sync.dma_start(
                                out=xtt[:].rearrange("p f t nt d -> p (f t nt d)"),
                                in_=xt_d[:].rearrange("p f t nt d -> p (f t nt d)"),
                            )
                            if mode != "dma":
                                vtt = vpool.tile(
                                    [tp, tq, T], mybir.dt.int32, tag="vT", bufs=2
                                )
                                ott = opool.tile(
                                    [tp, tq, NT, D], fdt, tag="oT", bufs=2
                                )
                                nc.sync.dma_start(
                                    out=vtt[:].rearrange("p f t -> p (f t)"),
                                    in_=vt_d[:],
                                )
                                nc.scalar.copy(
                                    ott[:],
                                    dflt[0:tp]
                                    .unsqueeze(1)
                                    .broadcast_to((tp, tq, NT, D)),
                                )
                        if k == 4 and ring == "v21" and mode != "dma":
                            compute(tp, tq, xtt, vtt[:], ott[:], "T")
                            nc.sync.dma_start(
                                out=ot_d[:],
                                in_=ott[:].rearrange("p f nt d -> p (f nt d)"),
                            )
                        if k == 4 and ring != "v21":
                            # tail pass mid-pipeline on sync
                            xtt = xpool.tile(
                                [tp, tq, T, NT, D], fdt, tag="xT", bufs=2
                            )
                            nc.sync.dma_start(
                                out=xtt[:].rearrange("p f t nt d -> p (f t nt d)"),
                                in_=xt_d[:].rearrange("p f t nt d -> p (f t nt d)"),
                            )
                            if mode != "dma":
                                vtt = vpool.tile(
                                    [tp, tq, T], mybir.dt.int32, tag="vT", bufs=2
                                )
                                ott = opool.tile(
                                    [tp, tq, NT, D], fdt, tag="oT", bufs=2
                                )
                                nc.sync.dma_start(
                                    out=vtt[:].rearrange("p f t -> p (f t)"),
                                    in_=vt_d[:],
                                )
                                nc.scalar.copy(
                                    ott[:],
                                    dflt[0:tp]
                                    .unsqueeze(1)
                                    .broadcast_to((tp, tq, NT, D)),
                                )
                                compute(tp, tq, xtt, vtt[:], ott[:], "T")
                                nc.sync.dma_start(
                                    out=ot_d[:],
                                    in_=ott[:].rearrange("p f nt d -> p (f nt d)"),
                                )
                        if mode == "dma":
                            if kk == 1:
                                nc.scalar.dma_start(
                                    out=om[j],
                                    in_=xt[:]
                                    .rearrange("p f t nt d -> p (f t nt d)")[
                                        :, 0 : 2 * q * NT * D
                                    ]
                                    .rearrange("p (n e) -> p n e", n=2),
                                )
                            continue
                        compute(P8, q, xt, vt[:, k], ots[j][:, kk], "")
                        steng = nc.sync if ring == "v21" else nc.scalar
                        if j < npair - 1:
                            if kk == 1:
                                steng.dma_start(
                                    out=om[j],
                                    in_=ots[j][:].rearrange(
                                        "p n f nt d -> p n (f nt d)"
                                    ),
                                )
                        else:
                            # last pair: store singles right after each blend
                            steng.dma_start(
                                out=om1[k],
                                in_=ots[j][:, kk].rearrange(
                                    "p f nt d -> p (f nt d)"
                                ),
                            )
        nc.compile()
        return nc

    if ring in ("v18", "v19"):
        # v7's cross-tile DVE interleave, rebuilt: v12's 3-op pair tree +
        # stt valid + 5 copy_predicated blends per tile, emitted as
        # round-robin streams (tile k-1's blends / tile k's tree / a slice
        # of the tail pass's tiny ops) so every serially-dependent DVE
        # pair is separated by independent work and the ~0.7us pipeline
        # drain stalls are hidden. Pending blends carry across the rep
        # boundary. v19 = v18 + x loads alternating gpsimd/sync queues.
        P8, q, nload = 128, 122, 8
        main = P8 * q * nload  # 124,928
        tp, tq = 8, 9          # tail 72 = 8 * 9
        assert main + tp * tq == bc
        npair = nload // 2
        xm = x[0:main].rearrange("(n p f) t nt d -> n p f t nt d", p=P8, f=q)
        vm = v[0:main].rearrange("(n p f) t -> p n (f t)", p=P8, f=q)
        om = o[0:main].rearrange("(j n p f) nt d -> j p n (f nt d)", p=P8, f=q, n=2)
        om1 = o[0:main].rearrange("(n p f) nt d -> n p (f nt d)", p=P8, f=q)
        xt_d = x[main:bc].rearrange("(p f) t nt d -> p f t nt d", p=tp, f=tq)
        vt_d = v[main:bc].rearrange("(p f) t -> p (f t)", p=tp, f=tq)
        ot_d = o[main:bc].rearrange("(p f) nt d -> p (f nt d)", p=tp, f=tq)
        with tile.TileContext(nc) as tc:
            with (
                tc.tile_pool(name="xs", bufs=3) as xpool,
                tc.tile_pool(name="vs", bufs=2) as vpool,
                tc.tile_pool(name="os", bufs=3) as opool,
                tc.tile_pool(name="wk", bufs=2) as wpool,
            ):
                dflt = wpool.tile([P8, NT, D], fdt, tag="dflt", bufs=1)
                nc.gpsimd.memset(dflt[:, :, 0:5], 0.0)
                nc.gpsimd.memset(dflt[:, :, 5:6], 1.0)

                def tree_ops(pp, ff, xt, vslice, a2, a, valid):
                    yield lambda: nc.vector.tensor_max(
                        a2[:], xt[:, :, :, :, 0:2], xt[:, :, :, :, 2:4]
                    )
                    yield lambda: nc.vector.tensor_max(
                        a[:], a2[:, :, :, :, 0], a2[:, :, :, :, 1]
                    )
                    yield lambda: nc.vector.tensor_max(a[:], a[:], xt[:, :, :, :, 4])
                    vb = vslice.unsqueeze(3).broadcast_to((pp, ff, T, NT))
                    yield lambda: nc.vector.scalar_tensor_tensor(
                        out=valid[:], in0=a[:], scalar=0.5, in1=vb,
                        op0=AluOpType.is_gt, op1=AluOpType.mult,
                    )

                def blend_ops(pp, ff, xt, valid, ot, post=None):
                    for t in range(T):
                        mask = (
                            valid[:, :, t, :]
                            .unsqueeze(3)
                            .broadcast_to((pp, ff, NT, D))
                        )
                        yield lambda m=mask, tt=t: _copy_predicated(
                            nc.vector, ot, m, xt[:, :, tt, :, :]
                        )
                    if post is not None:
                        yield post

                pending = []   # previous tile's blend ops (+ store trigger)
                tail_ops = []  # tail pass compute, drip-fed 2 ops per tile
                for r in range(reps):
                    vt = vpool.tile(
                        [P8, nload, q, T], mybir.dt.int32, tag="v",
                        bufs=1 if ring in ("v20", "v21") else 2,
                    )
                    ots = {}
                    for k in range(nload):
                        j, kk = k // 2, k % 2
                        if kk == 0 and mode != "dma":
                            for jj in [0, 1] if j == 0 else [j + 1]:
                                if jj >= npair:
                                    continue
                                opair = opool.tile(
                                    [P8, 2, q, NT, D], fdt, tag="o"
                                )
                                ots[jj] = opair
                                nc.scalar.copy(
                                    ots[jj][:],
                                    dflt[:]
                                    .unsqueeze(1)
                                    .unsqueeze(1)
                                    .broadcast_to((P8, 2, q, NT, D)),
                                )
                        xt = xpool.tile([P8, q, T, NT, D], fdt, tag="x")
                        xeng = nc.gpsimd if (ring == "v18" or k % 2 == 0) else nc.sync
                        xeng.dma_start(
                            out=xt[:].rearrange("p f t nt d -> p (f t nt d)"),
                            in_=xm[k].rearrange("p f t nt d -> p (f t nt d)"),
                        )
                        if k == 0:
                            nc.scalar.dma_start(
                                out=vt[:, 0:4].rearrange("p n f t -> p n (f t)"),
                                in_=vm[:, 0:4, :],
                            )
                        elif k == 2:
                            nc.scalar.dma_start(
                                out=vt[:, 4:8].rearrange("p n f t -> p n (f t)"),
                                in_=vm[:, 4:8, :],
                            )
                        if k == 0:
                            # tail DMAs up front on the idle sync queue
                            xtt = xpool.tile(
                                [tp, tq, T, NT, D], fdt, tag="xT", bufs=2
                            )
                            nc.sync.dma_start(
                                out=xtt[:].rearrange("p f t nt d -> p (f t nt d)"),
                                in_=xt_d[:].rearrange("p f t nt d -> p (f t nt d)"),
                            )
                            if mode != "dma":
                                vtt = vpool.tile(
                                    [tp, tq, T], mybir.dt.int32, tag="vT", bufs=2
                                )
                                ott = opool.tile(
                                    [tp, tq, NT, D], fdt, tag="oT", bufs=2
                                )
                                nc.sync.dma_start(
                                    out=vtt[:].rearrange("p f t -> p (f t)"),
                                    in_=vt_d[:],
                                )
                                nc.scalar.copy(
                                    ott[:],
                                    dflt[0:tp]
                                    .unsqueeze(1)
                                    .broadcast_to((tp, tq, NT, D)),
                                )
                                a2T = wpool.tile(
                                    [tp, tq, T, NT, 2], fdt, tag="a2T", bufs=2
                                )
                                aT = wpool.tile(
                                    [tp, tq, T, NT], fdt, tag="aT", bufs=2
                                )
                                vaT = wpool.tile(
                                    [tp, tq, T, NT], mybir.dt.uint8, tag="vaT",
                                    bufs=2,
                                )
                                tail_ops = list(
                                    tree_ops(tp, tq, xtt, vtt[:], a2T, aT, vaT)
                                ) + list(
                                    blend_ops(
                                        tp, tq, xtt, vaT, ott[:],
                                        post=lambda ott=ott: nc.sync.dma_start(
                                            out=ot_d[:],
                                            in_=ott[:].rearrange(
                                                "p f nt d -> p (f nt d)"
                                            ),
                                        ),
                                    )
                                )
                        if mode == "dma":
                            if kk == 1:
                                nc.scalar.dma_start(
                                    out=om[j],
                                    in_=xt[:]
                                    .rearrange("p f t nt d -> p (f t nt d)")[
                                        :, 0 : 2 * q * NT * D
                                    ]
                                    .rearrange("p (n e) -> p n e", n=2),
                                )
                            continue
                        a2 = wpool.tile([P8, q, T, NT, 2], fdt, tag="a2")
                        a = wpool.tile([P8, q, T, NT], fdt, tag="a")
                        valid = wpool.tile([P8, q, T, NT], mybir.dt.uint8, tag="va")
                        tree = list(tree_ops(P8, q, xt, vt[:, k], a2, a, valid))
                        chunk, tail_ops = tail_ops[:2], tail_ops[2:]
                        # round-robin: blend(k-1) / tree(k) / tail chunk
                        streams = [list(pending), tree, chunk]
                        while any(streams):
                            for s in streams:
                                if s:
                                    s.pop(0)()
                        ostore = None
                        if j < npair - 1:
                            if kk == 1:
                                # bind the tile now: ots is rebound per rep
                                # but this lambda fires next round
                                ostore = lambda jj=j, op=ots[j]: nc.scalar.dma_start(
                                    out=om[jj],
                                    in_=op[:].rearrange(
                                        "p n f nt d -> p n (f nt d)"
                                    ),
                                )
                        else:
                            # last pair: store singles so the final store
                            # starts one blend earlier
                            ostore = lambda jj=j, kkk=kk, op=ots[j]: nc.scalar.dma_start(
                                out=om1[2 * jj + kkk],
                                in_=op[:, kkk].rearrange(
                                    "p f nt d -> p (f nt d)"
                                ),
                            )
                        pending = list(
                            blend_ops(
                                P8, q, xt, valid, ots[j][:, kk], post=ostore
                            )
                        )
                if mode != "dma":
                    # drain: last tile's blends (+ any leftover tail ops)
                    streams = [pending, tail_ops]
                    while any(streams):
                        for s in streams:
                            if s:
                                s.pop(0)()
                    pending, tail_ops = [], []
        nc.compile()
        return nc

    if ring == "v7":
        # v2 with the DVE stream software-pipelined: tile k's max-tree
        # ops (a serially-dependent chain that otherwise stalls ~0.7us
        # per op on pipeline drains) are interleaved with tile k-1's
        # independent copy_predicated blend chain.
        P8, q, nload = 128, 122, 8
        main = P8 * q * nload  # 124,928
        tp, tq = 8, 9          # tail 72 = 8 * 9
        assert main + tp * tq == bc
        npair = nload // 2
        xm = x[0:main].rearrange("(n p f) t nt d -> n p f t nt d", p=P8, f=q)
        vm = v[0:main].rearrange("(n p f) t -> p n (f t)", p=P8, f=q)
        om = o[0:main].rearrange("(j n p f) nt d -> j p n (f nt d)", p=P8, f=q, n=2)
        xt_d = x[main:bc].rearrange("(p f) t nt d -> p f t nt d", p=tp, f=tq)
        vt_d = v[main:bc].rearrange("(p f) t -> p (f t)", p=tp, f=tq)
        ot_d = o[main:bc].rearrange("(p f) nt d -> p (f nt d)", p=tp, f=tq)
        with tile.TileContext(nc) as tc:
            with (
                tc.tile_pool(name="xs", bufs=3) as xpool,
                tc.tile_pool(name="vs", bufs=2) as vpool,
                tc.tile_pool(name="os", bufs=3) as opool,
                tc.tile_pool(name="wk", bufs=2) as wpool,
            ):
                dflt = wpool.tile([P8, NT, D], fdt, tag="dflt", bufs=1)
                nc.gpsimd.memset(dflt[:, :, 0:5], 0.0)
                nc.gpsimd.memset(dflt[:, :, 5:6], 1.0)

                def tree_ops(xt, vslice, a, bt, valid):
                    yield lambda: nc.vector.tensor_max(
                        a[:], xt[:, :, :, :, 0], xt[:, :, :, :, 1]
                    )
                    yield lambda: nc.vector.tensor_max(
                        bt[:], xt[:, :, :, :, 2], xt[:, :, :, :, 3]
                    )
                    yield lambda: nc.vector.tensor_max(a[:], a[:], bt[:])
                    yield lambda: nc.vector.tensor_max(a[:], a[:], xt[:, :, :, :, 4])
                    vb = vslice.unsqueeze(3).broadcast_to((P8, q, T, NT))
                    yield lambda: nc.vector.scalar_tensor_tensor(
                        out=valid[:], in0=a[:], scalar=0.5, in1=vb,
                        op0=AluOpType.is_gt, op1=AluOpType.mult,
                    )

                def blend_ops(xt, valid, ot):
                    for t in range(T):
                        mask = (
                            valid[:, :, t, :]
                            .unsqueeze(3)
                            .broadcast_to((P8, q, NT, D))
                        )
                        yield lambda m=mask, tt=t: _copy_predicated(
                            nc.vector, ot, m, xt[:, :, tt, :, :]
                        )

                def interleave(g1, g2):
                    l1, l2 = list(g1), list(g2)
                    n = max(len(l1), len(l2))
                    for idx in range(n):
                        if idx < len(l1):
                            l1[idx]()
                        if idx < len(l2):
                            l2[idx]()

                for r in range(reps):
                    # tail pass on sync; compute on DVE ahead of the pipe
                    xtt = xpool.tile([tp, tq, T, NT, D], fdt, tag="xT", bufs=2)
                    nc.sync.dma_start(
                        out=xtt[:].rearrange("p f t nt d -> p (f t nt d)"),
                        in_=xt_d[:].rearrange("p f t nt d -> p (f t nt d)"),
                    )
                    if mode != "dma":
                        vtt = vpool.tile([tp, tq, T], mybir.dt.int32, tag="vT", bufs=2)
                        ott = opool.tile([tp, tq, NT, D], fdt, tag="oT", bufs=2)
                        nc.sync.dma_start(
                            out=vtt[:].rearrange("p f t -> p (f t)"), in_=vt_d[:]
                        )
                        nc.scalar.copy(
                            ott[:],
                            dflt[0:tp].unsqueeze(1).broadcast_to((tp, tq, NT, D)),
                        )
                        aT = wpool.tile([tp, tq, T, NT], fdt, tag="aT")
                        bT = wpool.tile([tp, tq, T, NT], fdt, tag="bT")
                        vaT = wpool.tile([tp, tq, T, NT], mybir.dt.uint8, tag="vaT")
                        nc.vector.tensor_max(
                            aT[:], xtt[:, :, :, :, 0], xtt[:, :, :, :, 1]
                        )
                        nc.vector.tensor_max(
                            bT[:], xtt[:, :, :, :, 2], xtt[:, :, :, :, 3]
                        )
                        nc.vector.tensor_max(aT[:], aT[:], bT[:])
                        nc.vector.tensor_max(aT[:], aT[:], xtt[:, :, :, :, 4])
                        vbT = vtt[:].unsqueeze(3).broadcast_to((tp, tq, T, NT))
                        nc.vector.scalar_tensor_tensor(
                            out=vaT[:], in0=aT[:], scalar=0.5, in1=vbT,
                            op0=AluOpType.is_gt, op1=AluOpType.mult,
                        )
                        for t in range(T):
                            maskT = (
                                vaT[:, :, t, :]
                                .unsqueeze(3)
                                .broadcast_to((tp, tq, NT, D))
                            )
                            _copy_predicated(
                                nc.vector, ott[:], maskT, xtt[:, :, t, :, :]
                            )
                        nc.sync.dma_start(
                            out=ot_d[:],
                            in_=ott[:].rearrange("p f nt d -> p (f nt d)"),
                        )
                    vt = vpool.tile([P8, nload, q, T], mybir.dt.int32, tag="v")
                    nc.scalar.dma_start(
                        out=vt[:].rearrange("p n f t -> p n (f t)"), in_=vm
                    )
                    ots = {}
                    xts = {}
                    valids = {}

                    def emit_store(kb):
                        jb = kb // 2
                        nc.scalar.dma_start(
                            out=om[jb],
                            in_=ots[jb][:].rearrange("p n f nt d -> p n (f nt d)"),
                        )

                    for k in range(nload):
                        j, kk = k // 2, k % 2
                        if kk == 0 and mode != "dma":
                            for jj in [0, 1] if j == 0 else [j + 1]:
                                if jj >= npair:
                                    continue
                                opair = opool.tile(
                                    [P8, 2, q, NT, D], fdt, tag="o"
                                )
                                ots[jj] = opair
                                nc.scalar.copy(
                                    ots[jj][:],
                                    dflt[:]
                                    .unsqueeze(1)
                                    .unsqueeze(1)
                                    .broadcast_to((P8, 2, q, NT, D)),
                                )
                        xt = xpool.tile([P8, q, T, NT, D], fdt, tag="x")
                        nc.gpsimd.dma_start(
                            out=xt[:].rearrange("p f t nt d -> p (f t nt d)"),
                            in_=xm[k].rearrange("p f t nt d -> p (f t nt d)"),
                        )
                        xts[k] = xt
                        if mode == "dma":
                            if kk == 1:
                                nc.scalar.dma_start(
                                    out=om[j],
                                    in_=xt[:]
                                    .rearrange("p f t nt d -> p (f t nt d)")[
                                        :, 0 : 2 * q * NT * D
                                    ]
                                    .rearrange("p (n e) -> p n e", n=2),
                                )
                            continue
                        a = wpool.tile([P8, q, T, NT], fdt, tag="a")
                        bt = wpool.tile([P8, q, T, NT], fdt, tag="b")
                        valid = wpool.tile([P8, q, T, NT], mybir.dt.uint8, tag="va")
                        valids[k] = valid
                        tree = tree_ops(xt, vt[:, k], a, bt, valid)
                        if k == 0:
                            interleave(tree, [])
                        else:
                            kb = k - 1
                            jb, kkb = kb // 2, kb % 2
                            interleave(
                                tree,
                                blend_ops(xts[kb], valids[kb], ots[jb][:, kkb]),
                            )
                            if kkb == 1:
                                emit_store(kb)
                    if mode != "dma":
                        kb = nload - 1
                        jb, kkb = kb // 2, kb % 2
                        interleave([], blend_ops(xts[kb], valids[kb], ots[jb][:, kkb]))
                        emit_store(kb)
        nc.compile()
        return nc

    if ring == "v5":
        # v2 with the valid-computation (max tree + threshold) moved to
        # GPSIMD, interleaved with its x-load descriptor pushes (lagged
        # one tile so loads stay ahead). DVE runs only the 5
        # copy_predicated blends (~7.6us/tile); every engine then sits
        # under the ~106us/core DMA floor.
        P8, q, nload = 128, 122, 8
        main = P8 * q * nload  # 124,928
        tp, tq = 8, 9          # tail 72 = 8 * 9
        assert main + tp * tq == bc
        npair = nload // 2
        xm = x[0:main].rearrange("(n p f) t nt d -> n p f t nt d", p=P8, f=q)
        vm = v[0:main].rearrange("(n p f) t -> p n (f t)", p=P8, f=q)
        om = o[0:main].rearrange("(j n p f) nt d -> j p n (f nt d)", p=P8, f=q, n=2)
        xt_d = x[main:bc].rearrange("(p f) t nt d -> p f t nt d", p=tp, f=tq)
        vt_d = v[main:bc].rearrange("(p f) t -> p (f t)", p=tp, f=tq)
        ot_d = o[main:bc].rearrange("(p f) nt d -> p (f nt d)", p=tp, f=tq)
        with tile.TileContext(nc) as tc:
            with (
                tc.tile_pool(name="xs", bufs=3) as xpool,
                tc.tile_pool(name="vs", bufs=2) as vpool,
                tc.tile_pool(name="os", bufs=3) as opool,
                tc.tile_pool(name="wk", bufs=2) as wpool,
            ):
                dflt = wpool.tile([P8, NT, D], fdt, tag="dflt", bufs=1)
                nc.gpsimd.memset(dflt[:, :, 0:5], 0.0)
                nc.gpsimd.memset(dflt[:, :, 5:6], 1.0)

                def valid_of(pp, ff, xt, vslice, tg, eng):
                    a = wpool.tile([pp, ff, T, NT], fdt, tag="a" + tg)
                    bt = wpool.tile([pp, ff, T, NT], fdt, tag="b" + tg)
                    valid = wpool.tile(
                        [pp, ff, T, NT], mybir.dt.uint8, tag="va" + tg
                    )
                    eng.tensor_max(a[:], xt[:, :, :, :, 0], xt[:, :, :, :, 1])
                    eng.tensor_max(bt[:], xt[:, :, :, :, 2], xt[:, :, :, :, 3])
                    eng.tensor_max(a[:], a[:], bt[:])
                    eng.tensor_max(a[:], a[:], xt[:, :, :, :, 4])
                    vb = vslice.unsqueeze(3).broadcast_to((pp, ff, T, NT))
                    eng.scalar_tensor_tensor(
                        out=valid[:], in0=a[:], scalar=0.5, in1=vb,
                        op0=AluOpType.is_gt, op1=AluOpType.mult,
                    )
                    return valid

                def blend(pp, ff, xt, valid, ot):
                    for t in range(T):
                        mask = (
                            valid[:, :, t, :]
                            .unsqueeze(3)
                            .broadcast_to((pp, ff, NT, D))
                        )
                        _copy_predicated(nc.vector, ot, mask, xt[:, :, t, :, :])

                for r in range(reps):
                    # tail pass on sync; its compute fully on DVE
                    xtt = xpool.tile([tp, tq, T, NT, D], fdt, tag="xT", bufs=2)
                    nc.sync.dma_start(
                        out=xtt[:].rearrange("p f t nt d -> p (f t nt d)"),
                        in_=xt_d[:].rearrange("p f t nt d -> p (f t nt d)"),
                    )
                    vt = vpool.tile([P8, nload, q, T], mybir.dt.int32, tag="v")
                    nc.scalar.dma_start(
                        out=vt[:].rearrange("p n f t -> p n (f t)"), in_=vm
                    )
                    if mode != "dma":
                        vtt = vpool.tile([tp, tq, T], mybir.dt.int32, tag="vT", bufs=2)
                        ott = opool.tile([tp, tq, NT, D], fdt, tag="oT", bufs=2)
                        nc.sync.dma_start(
                            out=vtt[:].rearrange("p f t -> p (f t)"), in_=vt_d[:]
                        )
                        nc.scalar.copy(
                            ott[:],
                            dflt[0:tp].unsqueeze(1).broadcast_to((tp, tq, NT, D)),
                        )
                        validT = valid_of(tp, tq, xtt, vtt[:], "T", nc.vector)
                        blend(tp, tq, xtt, validT, ott[:])
                        nc.sync.dma_start(
                            out=ot_d[:],
                            in_=ott[:].rearrange("p f nt d -> p (f nt d)"),
                        )
                    ots = {}
                    pend = []  # (k, xt) awaiting gpsimd valid-compute
                    xts = {}
                    valids = {}
                    for k in range(nload):
                        j, kk = k // 2, k % 2
                        if kk == 0 and mode != "dma":
                            for jj in [0, 1] if j == 0 else [j + 1]:
                                if jj >= npair:
                                    continue
                                opair = opool.tile(
                                    [P8, 2, q, NT, D], fdt, tag="o"
                                )
                                ots[jj] = opair
                                nc.scalar.copy(
                                    ots[jj][:],
                                    dflt[:]
                                    .unsqueeze(1)
                                    .unsqueeze(1)
                                    .broadcast_to((P8, 2, q, NT, D)),
                                )
                        xt = xpool.tile([P8, q, T, NT, D], fdt, tag="x")
                        nc.gpsimd.dma_start(
                            out=xt[:].rearrange("p f t nt d -> p (f t nt d)"),
                            in_=xm[k].rearrange("p f t nt d -> p (f t nt d)"),
                        )
                        xts[k] = xt
                        if mode == "dma":
                            if kk == 1:
                                nc.scalar.dma_start(
                                    out=om[j],
                                    in_=xt[:]
                                    .rearrange("p f t nt d -> p (f t nt d)")[
                                        :, 0 : 2 * q * NT * D
                                    ]
                                    .rearrange("p (n e) -> p n e", n=2),
                                )
                            continue
                        pend.append(k)
                        if k > 0:
                            kv = pend.pop(0)
                            valids[kv] = valid_of(
                                P8, q, xts[kv], vt[:, kv], "", nc.gpsimd
                            )
                        if k == nload - 1:
                            kv = pend.pop(0)
                            valids[kv] = valid_of(
                                P8, q, xts[kv], vt[:, kv], "", nc.gpsimd
                            )
                        # DVE blends for any tile whose valid is ready
                        for kb in [k - 1, k] if k == nload - 1 else [k - 1]:
                            if kb < 0 or kb not in valids:
                                continue
                            jb, kkb = kb // 2, kb % 2
                            blend(P8, q, xts[kb], valids[kb], ots[jb][:, kkb])
                            if kkb == 1:
                                nc.scalar.dma_start(
                                    out=om[jb],
                                    in_=ots[jb][:].rearrange(
                                        "p n f nt d -> p n (f nt d)"
                                    ),
                                )
        nc.compile()
        return nc

    if ring in ("v2", "v4", "v6", "v10", "v11", "v16"):
        # k128 + DMA batching: ONE v-load per rep ([128, 8, 610] i32,
        # 2.5 MB), o-stores merged per tile-pair ([128, 2, 122, 2, 6],
        # 1.5 MB), output init copied one pair ahead (opool bufs=3) so
        # the ACT stream's store-waits don't stall the next pair's init.
        # Compute variants: v4 = pool_max hm; v6 = max tree via
        # TensorScalarPtr; v10 = contiguous-pair max tree; v11 = v10 +
        # single-instruction blend (stride-0-t out AP).
        P8, q, nload = 128, 122, 8
        main = P8 * q * nload  # 124,928
        tp, tq = 8, 9          # tail 72 = 8 * 9
        assert main + tp * tq == bc
        npair = nload // 2
        xm = x[0:main].rearrange("(n p f) t nt d -> n p f t nt d", p=P8, f=q)
        vm = v[0:main].rearrange("(n p f) t -> p n (f t)", p=P8, f=q)
        om = o[0:main].rearrange("(j n p f) nt d -> j p n (f nt d)", p=P8, f=q, n=2)
        xt_d = x[main:bc].rearrange("(p f) t nt d -> p f t nt d", p=tp, f=tq)
        vt_d = v[main:bc].rearrange("(p f) t -> p (f t)", p=tp, f=tq)
        ot_d = o[main:bc].rearrange("(p f) nt d -> p (f nt d)", p=tp, f=tq)
        with tile.TileContext(nc) as tc:
            with (
                tc.tile_pool(name="xs", bufs=3) as xpool,
                tc.tile_pool(name="vs", bufs=2) as vpool,
                tc.tile_pool(name="os", bufs=3) as opool,
                tc.tile_pool(name="wk", bufs=2) as wpool,
            ):
                dflt = wpool.tile([P8, NT, D], fdt, tag="dflt", bufs=1)
                nc.gpsimd.memset(dflt[:, :, 0:5], 0.0)
                nc.gpsimd.memset(dflt[:, :, 5:6], 1.0)
                nbias = None
                if ring in ("v20", "v21"):
                    nbias = wpool.tile([P8, 1], fdt, tag="nbias", bufs=1)
                    nc.gpsimd.memset(nbias[:], -0.5)

                def compute(pp, ff, xt, vslice, ot, tg):
                    a = wpool.tile([pp, ff, T, NT], fdt, tag="a" + tg)
                    valid = wpool.tile(
                        [pp, ff, T, NT], mybir.dt.uint8, tag="va" + tg
                    )
                    if ring == "v4":
                        nc.vector.pool_max(a[:], xt[:, :, :, :, 0:5])
                    elif ring in ("v6", "v16"):
                        bt = wpool.tile([pp, ff, T, NT], fdt, tag="b" + tg)

                        def stt_max(out, i0, i1):
                            nc.vector.scalar_tensor_tensor(
                                out=out, in0=i0, scalar=1.0, in1=i1,
                                op0=AluOpType.mult, op1=AluOpType.max,
                            )

                        stt_max(a[:], xt[:, :, :, :, 0], xt[:, :, :, :, 1])
                        stt_max(bt[:], xt[:, :, :, :, 2], xt[:, :, :, :, 3])
                        stt_max(a[:], a[:], bt[:])
                        stt_max(a[:], a[:], xt[:, :, :, :, 4])
                    elif ring in ("v10", "v11"):
                        # level 1 reads contiguous 2-runs (pairs (0,2),
                        # (1,3)); only levels 2/3 touch strided APs
                        a2 = wpool.tile([pp, ff, T, NT, 2], fdt, tag="a2" + tg)
                        nc.vector.tensor_max(
                            a2[:], xt[:, :, :, :, 0:2], xt[:, :, :, :, 2:4]
                        )
                        nc.vector.tensor_max(
                            a[:], a2[:, :, :, :, 0], a2[:, :, :, :, 1]
                        )
                        nc.vector.tensor_max(a[:], a[:], xt[:, :, :, :, 4])
                    else:
                        bt = wpool.tile([pp, ff, T, NT], fdt, tag="b" + tg)
                        nc.vector.tensor_max(
                            a[:], xt[:, :, :, :, 0], xt[:, :, :, :, 1]
                        )
                        nc.vector.tensor_max(
                            bt[:], xt[:, :, :, :, 2], xt[:, :, :, :, 3]
                        )
                        nc.vector.tensor_max(a[:], a[:], bt[:])
                        nc.vector.tensor_max(a[:], a[:], xt[:, :, :, :, 4])
                    vb = vslice.unsqueeze(3).broadcast_to((pp, ff, T, NT))
                    nc.vector.scalar_tensor_tensor(
                        out=valid[:], in0=a[:], scalar=0.5, in1=vb,
                        op0=AluOpType.is_gt, op1=AluOpType.mult,
                    )
                    if ring in ("v11", "v16"):
                        # single-instruction blend: out AP broadcast over
                        # t (stride 0); ascending-t same-address writes
                        # commit in order, so the last valid t wins
                        maskT = valid[:].unsqueeze(4).broadcast_to(
                            (pp, ff, T, NT, D)
                        )
                        outT = ot.unsqueeze(2).broadcast_to((pp, ff, T, NT, D))
                        _copy_predicated(nc.vector, outT, maskT, xt[:])
                    else:
                        for t in range(T):
                            mask = (
                                valid[:, :, t, :]
                                .unsqueeze(3)
                                .broadcast_to((pp, ff, NT, D))
                            )
                            _copy_predicated(
                                nc.vector, ot, mask, xt[:, :, t, :, :]
                            )

                if mode.startswith("comp"):
                    # compute-only: one resident x/v tile, 8 compute chains
                    # per rep, single store at the end. "comp"=full chain,
                    # "compM"=maxes+stt only, "compC"=cpreds only,
                    # "compI"=init copies only.
                    xt = xpool.tile([P8, q, T, NT, D], fdt, tag="x")
                    nc.gpsimd.dma_start(
                        out=xt[:].rearrange("p f t nt d -> p (f t nt d)"),
                        in_=xm[0].rearrange("p f t nt d -> p (f t nt d)"),
                    )
                    vt = vpool.tile([P8, nload, q, T], mybir.dt.int32, tag="v")
                    nc.scalar.dma_start(
                        out=vt[:].rearrange("p n f t -> p n (f t)"), in_=vm
                    )
                    vfix = wpool.tile([P8, q, T, NT], mybir.dt.uint8, tag="vfix", bufs=1)
                    nc.gpsimd.memset(vfix[:], 1)
                    for r in range(reps):
                        for k in range(nload):
                            opair = opool.tile([P8, 2, q, NT, D], fdt, tag="o")
                            if mode in ("comp", "compI"):
                                nc.scalar.copy(
                                    opair[:],
                                    dflt[:]
                                    .unsqueeze(1)
                                    .unsqueeze(1)
                                    .broadcast_to((P8, 2, q, NT, D)),
                                )
                            if mode == "comp":
                                compute(P8, q, xt, vt[:, k], opair[:, k % 2], "")
                            elif mode == "compM":
                                a = wpool.tile([P8, q, T, NT], fdt, tag="a")
                                bt = wpool.tile([P8, q, T, NT], fdt, tag="b")
                                valid = wpool.tile(
                                    [P8, q, T, NT], mybir.dt.uint8, tag="va"
                                )
                                nc.vector.tensor_max(
                                    a[:], xt[:, :, :, :, 0], xt[:, :, :, :, 1]
                                )
                                nc.vector.tensor_max(
                                    bt[:], xt[:, :, :, :, 2], xt[:, :, :, :, 3]
                                )
                                nc.vector.tensor_max(a[:], a[:], bt[:])
                                nc.vector.tensor_max(a[:], a[:], xt[:, :, :, :, 4])
                                vb = vt[:, k].unsqueeze(3).broadcast_to(
                                    (P8, q, T, NT)
                                )
                                nc.vector.scalar_tensor_tensor(
                                    out=valid[:], in0=a[:], scalar=0.5, in1=vb,
                                    op0=AluOpType.is_gt, op1=AluOpType.mult,
                                )
                            elif mode == "compC":
                                ot = opair[:, k % 2]
                                for t in range(T):
                                    mask = (
                                        vfix[:, :, t, :]
                                        .unsqueeze(3)
                                        .broadcast_to((P8, q, NT, D))
                                    )
                                    _copy_predicated(
                                        nc.vector, ot, mask, xt[:, :, t, :, :]
                                    )
                    if mode == "compM":
                        nc.scalar.dma_start(
                            out=om[0],
                            in_=xt[:]
                            .rearrange("p f t nt d -> p (f t nt d)")[
                                :, 0 : 2 * q * NT * D
                            ]
                            .rearrange("p (n e) -> p n e", n=2),
                        )
                    else:
                        nc.scalar.dma_start(
                            out=om[0],
                            in_=opair[:].rearrange("p n f nt d -> p n (f nt d)"),
                        )

                for r in range(reps if not mode.startswith("comp") else 0):
                    # tail pass on sync (off the hot queues)
                    xtt = xpool.tile([tp, tq, T, NT, D], fdt, tag="xT", bufs=2)
                    nc.sync.dma_start(
                        out=xtt[:].rearrange("p f t nt d -> p (f t nt d)"),
                        in_=xt_d[:].rearrange("p f t nt d -> p (f t nt d)"),
                    )
                    vt = vpool.tile([P8, nload, q, T], mybir.dt.int32, tag="v")
                    nc.scalar.dma_start(
                        out=vt[:].rearrange("p n f t -> p n (f t)"), in_=vm
                    )
                    if mode != "dma":
                        vtt = vpool.tile([tp, tq, T], mybir.dt.int32, tag="vT", bufs=2)
                        ott = opool.tile([tp, tq, NT, D], fdt, tag="oT", bufs=2)
                        nc.sync.dma_start(
                            out=vtt[:].rearrange("p f t -> p (f t)"), in_=vt_d[:]
                        )
                        nc.scalar.copy(
                            ott[:],
                            dflt[0:tp].unsqueeze(1).broadcast_to((tp, tq, NT, D)),
                        )
                        compute(tp, tq, xtt, vtt[:], ott[:], "T")
                        nc.sync.dma_start(
                            out=ot_d[:],
                            in_=ott[:].rearrange("p f nt d -> p (f nt d)"),
                        )
                    ots = {}
                    for k in range(nload):
                        j, kk = k // 2, k % 2
                        if kk == 0:
                            # init this pair's (and pair 0: also next's)
                            # output tile ahead of the store-wait
                            for jj in [0, 1] if j == 0 else [j + 1]:
                                if jj >= npair or mode == "dma":
                                    continue
                                opair = opool.tile(
                                    [P8, 2, q, NT, D], fdt, tag="o"
                                )
                                ots[jj] = opair
                                nc.scalar.copy(
                                    ots[jj][:],
                                    dflt[:]
                                    .unsqueeze(1)
                                    .unsqueeze(1)
                                    .broadcast_to((P8, 2, q, NT, D)),
                                )
                        xt = xpool.tile([P8, q, T, NT, D], fdt, tag="x")
                        nc.gpsimd.dma_start(
                            out=xt[:].rearrange("p f t nt d -> p (f t nt d)"),
                            in_=xm[k].rearrange("p f t nt d -> p (f t nt d)"),
                        )
                        if mode == "dma":
                            if kk == 1:
                                nc.scalar.dma_start(
                                    out=om[j],
                                    in_=xt[:]
                                    .rearrange("p f t nt d -> p (f t nt d)")[
                                        :, 0 : 2 * q * NT * D
                                    ]
                                    .rearrange("p (n e) -> p n e", n=2),
                                )
                            continue
                        compute(P8, q, xt, vt[:, k], ots[j][:, kk], "")
                        if kk == 1:
                            nc.scalar.dma_start(
                                out=om[j],
                                in_=ots[j][:].rearrange(
                                    "p n f nt d -> p n (f nt d)"
                                ),
                            )
        nc.compile()
        return nc

    if ring == "k128":
        # 128-partition DMA layout. DMA BW here is ~2.2x higher for
        # [128, chunk] transfers than [125, chunk] (348-361 GB/s vs
        # 154-179 GB/s measured), so split bc = 128*122*8 + 72: eight
        # [128, 122, T, NT, D] main tiles (x via gpsimd/SWDGE, v + out
        # via scalar/ACT HWDGE) plus one [8, 9, ...] tail pass on the
        # otherwise-idle sync queue.
        P8, q, nload = 128, 122, 8
        main = P8 * q * nload  # 124,928
        tp, tq = 8, 9          # tail 72 = 8 * 9
        assert main + tp * tq == bc
        xm = x[0:main].rearrange("(n p f) t nt d -> n p f t nt d", p=P8, f=q)
        vm = v[0:main].rearrange("(n p f) t -> n p (f t)", p=P8, f=q)
        om = o[0:main].rearrange("(n p f) nt d -> n p (f nt d)", p=P8, f=q)
        xt_d = x[main:bc].rearrange("(p f) t nt d -> p f t nt d", p=tp, f=tq)
        vt_d = v[main:bc].rearrange("(p f) t -> p (f t)", p=tp, f=tq)
        ot_d = o[main:bc].rearrange("(p f) nt d -> p (f nt d)", p=tp, f=tq)
        with tile.TileContext(nc) as tc:
            with (
                tc.tile_pool(name="xs", bufs=3) as xpool,
                tc.tile_pool(name="vs", bufs=2) as vpool,
                tc.tile_pool(name="os", bufs=2) as opool,
                tc.tile_pool(name="wk", bufs=2) as wpool,
            ):
                dflt = wpool.tile([P8, NT, D], fdt, tag="dflt", bufs=1)
                nc.gpsimd.memset(dflt[:, :, 0:5], 0.0)
                nc.gpsimd.memset(dflt[:, :, 5:6], 1.0)

                def compute(pp, ff, xt, vt, ot, tg):
                    a = wpool.tile([pp, ff, T, NT], fdt, tag="a" + tg)
                    bt = wpool.tile([pp, ff, T, NT], fdt, tag="b" + tg)
                    valid = wpool.tile(
                        [pp, ff, T, NT], mybir.dt.uint8, tag="va" + tg
                    )
                    nc.vector.tensor_max(a[:], xt[:, :, :, :, 0], xt[:, :, :, :, 1])
                    nc.vector.tensor_max(bt[:], xt[:, :, :, :, 2], xt[:, :, :, :, 3])
                    nc.vector.tensor_max(a[:], a[:], bt[:])
                    nc.vector.tensor_max(a[:], a[:], xt[:, :, :, :, 4])
                    vb = vt[:].unsqueeze(3).broadcast_to((pp, ff, T, NT))
                    nc.vector.scalar_tensor_tensor(
                        out=valid[:], in0=a[:], scalar=0.5, in1=vb,
                        op0=AluOpType.is_gt, op1=AluOpType.mult,
                    )
                    for t in range(T):
                        mask = (
                            valid[:, :, t, :]
                            .unsqueeze(3)
                            .broadcast_to((pp, ff, NT, D))
                        )
                        _copy_predicated(nc.vector, ot[:], mask, xt[:, :, t, :, :])

                for r in range(reps):
                    # tail pass: 72 elems on sync so its small DMAs stay
                    # off the two hot queues
                    xtt = xpool.tile([tp, tq, T, NT, D], fdt, tag="xT", bufs=2)
                    nc.sync.dma_start(
                        out=xtt[:].rearrange("p f t nt d -> p (f t nt d)"),
                        in_=xt_d[:].rearrange("p f t nt d -> p (f t nt d)"),
                    )
                    if mode != "dma":
                        vtt = vpool.tile([tp, tq, T], mybir.dt.int32, tag="vT", bufs=2)
                        ott = opool.tile([tp, tq, NT, D], fdt, tag="oT", bufs=2)
                        nc.sync.dma_start(
                            out=vtt[:].rearrange("p f t -> p (f t)"), in_=vt_d[:]
                        )
                        nc.scalar.copy(
                            ott[:],
                            dflt[0:tp].unsqueeze(1).broadcast_to((tp, tq, NT, D)),
                        )
                        compute(tp, tq, xtt, vtt, ott, "T")
                        nc.sync.dma_start(
                            out=ot_d[:],
                            in_=ott[:].rearrange("p f nt d -> p (f nt d)"),
                        )
                    for k in range(nload):
                        xt = xpool.tile([P8, q, T, NT, D], fdt, tag="x")
                        nc.gpsimd.dma_start(
                            out=xt[:].rearrange("p f t nt d -> p (f t nt d)"),
                            in_=xm[k].rearrange("p f t nt d -> p (f t nt d)"),
                        )
                        vt = vpool.tile([P8, q, T], mybir.dt.int32, tag="v")
                        nc.scalar.dma_start(
                            out=vt[:].rearrange("p f t -> p (f t)"), in_=vm[k]
                        )
                        if mode == "dma":
                            nc.scalar.dma_start(
                                out=om[k],
                                in_=xt[:].rearrange("p f t nt d -> p (f t nt d)")[
                                    :, 0 : q * NT * D
                                ],
                            )
                            continue
                        ot = opool.tile([P8, q, NT, D], fdt, tag="o")
                        nc.scalar.copy(
                            ot[:], dflt[:].unsqueeze(1).broadcast_to((P8, q, NT, D))
                        )
                        compute(P8, q, xt, vt, ot, "")
                        nc.scalar.dma_start(
                            out=om[k], in_=ot[:].rearrange("p f nt d -> p (f nt d)")
                        )
        nc.compile()
